# revision 1
# baseline (speedup 1.0000x reference)
"""DifferentialAttention Trainium2 kernel (8 NeuronCores, SPMD).

Sharding: data-parallel over batch B=4, tensor-parallel over heads
(2 cores per batch element, 8 heads each).  Each core computes the
partial projection output for its 8 heads; the host sums the two
partials per batch element and adds b_proj.

Per-core device pipeline (all matmuls bf16 inputs, fp32 PSUM accum):
  1. QKV^T = W_slice^T.T @ x^T            -> [channels, n] layout
  2. V transpose via PE (keys on partitions), ones column appended
  3. scores S^T[m, n] per (head, half) with 4-way row-group packing
     (contraction d=32 -> PE row groups 0/32/64/96)
  4. exp on ScalarE (scale=1/8 folded in), bf16 out
  5. PV:  [V | 1]^T @ E  -> unnormalized out^T + softmax denominator row
  6. combine: O^T = O1/d1 - lam*O2/d2 (reciprocal + GPSIMD partition
     broadcast + DVE mul/add)
  7. proj: out = O^T.T @ Wp_slice
"""

import sys

sys.path.insert(0, "/opt/trn_rl_repo")

import numpy as np
import ml_dtypes

B, N, C, H, HD = 4, 1024, 1024, 16, 64
LAMBDA_INIT = 0.8
BF16 = ml_dtypes.bfloat16

_PROG_CACHE = {}


def _build_program(loop_n=1, dma_outside=False, skip=()):
    key = ("nc", loop_n, dma_outside, tuple(skip))
    if key in _PROG_CACHE:
        return _PROG_CACHE[key]

    import contextlib

    import concourse.mybir as mybir
    import concourse.tile as tile
    from concourse import bacc

    f32 = mybir.dt.float32
    b16 = mybir.dt.bfloat16
    Exp = mybir.ActivationFunctionType.Exp

    nc = bacc.Bacc(None)

    x_d = nc.dram_tensor("xT", [8, 128, N], b16, kind="ExternalInput")
    wqkv_d = nc.dram_tensor("wqkv", [8, 128, 1536], b16, kind="ExternalInput")
    wp_d = nc.dram_tensor("wp", [4, 128, C], b16, kind="ExternalInput")
    neglam_d = nc.dram_tensor("neglam", [1, 1], f32, kind="ExternalInput")
    out_d = nc.dram_tensor("out", [8, 128, C], f32, kind="ExternalOutput")

    with tile.TileContext(nc) as tc:
        with (
            tc.tile_pool(name="io", bufs=1) as iopool,
            tc.tile_pool(name="work", bufs=4) as wpool,
            tc.tile_pool(name="esb", bufs=6) as epool,
            tc.tile_pool(name="pS", bufs=2, space="PSUM") as pS,
            tc.tile_pool(name="pV", bufs=1, space="PSUM") as pV,
        ):
            xT = iopool.tile([128, 8, N], b16)
            wqkv = iopool.tile([128, 8, 1536], b16)
            wp = iopool.tile([128, 4, C], b16)
            neglam = iopool.tile([1, 1], f32)
            ones1 = iopool.tile([1, 64], b16)
            # qkvT chunk j: j in 0..3 -> q head-pair j; 4..7 -> k head-pair
            # within a chunk: partitions 0-63 even head (d 0..63), 64-127 odd
            qkvT = iopool.tile([128, 8, N], b16)
            # V in [keys, channels] layout: [m%128, m//128, head_local, d|1]
            # col 64 of each head's block is the ones column (softmax denom)
            vsb = iopool.tile([128, 8, 8, 65], b16)
            # combined attention output (transposed): [cl, hp, n]
            oT = iopool.tile([128, 4, N], b16)

            nc.gpsimd.memset(ones1[:], 1.0)
            nc.gpsimd.memset(vsb[:, :, :, 64:65], 1.0)
            if "attn" in skip:
                nc.gpsimd.memset(oT[:], 0.0)

            def dma_inputs():
                # split across both HWDGE rings (SP + ACT) for 2x issue width
                for cc in range(8):
                    nc.sync.dma_start(xT[:, cc, :], x_d[cc])
                    nc.scalar.dma_start(wqkv[:, cc, :], wqkv_d[cc])
                for ci in range(4):
                    nc.sync.dma_start(wp[:, ci, :], wp_d[ci])
                nc.sync.dma_start(neglam[:], neglam_d[:])

            def qkv_psum():
                # phases A/C borrow score-pool slots ([128,2,512] granularity)
                t = pS.tile([128, 2, 512], f32, tag="s", name="qkvps")
                return t[:, 0, :]

            if dma_outside:
                dma_inputs()
            loop_ctx = (
                tc.For_i(0, loop_n, 1) if loop_n > 1 else contextlib.nullcontext()
            )
            loop_ctx.__enter__()
            if not dma_outside:
                dma_inputs()

            # ---------------- Phase A: QKV projection -----------------
            def emit_qkv_unit(hp, t, nh, on_act=True):
                # one [128,512] q/k projection chunk; evacuation on ScalarE
                # when ACT is idle (phase A), DVE inside the attention stream
                evac = nc.scalar.copy if on_act else nc.vector.tensor_copy
                j = t * 4 + hp
                ps = qkv_psum()
                for cc in range(8):
                    nc.tensor.matmul(
                        ps[:],
                        wqkv[:, cc, j * 128 : (j + 1) * 128],
                        xT[:, cc, nh * 512 : (nh + 1) * 512],
                        start=(cc == 0),
                        stop=(cc == 7),
                    )
                evac(qkvT[:, j, nh * 512 : (nh + 1) * 512], ps[:])

            def emit_qkv_qk(hp, on_act=True):
                for t in range(2):  # q, k -> [channels, n] layout
                    for nh in range(2):
                        emit_qkv_unit(hp, t, nh, on_act)

            def emit_v():
                # v -> [keys, channels] layout (operands swapped)
                for mc in range(8):
                    ps = qkv_psum()
                    for cc in range(8):
                        nc.tensor.matmul(
                            ps[:],
                            xT[:, cc, mc * 128 : (mc + 1) * 128],
                            wqkv[:, cc, 1024:1536],
                            start=(cc == 0),
                            stop=(cc == 7),
                        )
                    nc.scalar.copy(
                        vsb[:, mc, :, 0:64], ps.rearrange("p (g d) -> p g d", g=8)
                    )

            if "qkv" not in skip:
                emit_v()
                emit_qkv_qk(0)

            # ---------------- Phase B: attention ----------------------
            # combo order ci: 0=(even,h1) 1=(odd,h1) 2=(even,h2) 3=(odd,h2)
            # (rg, parity): rg = score row-group, parity selects V head
            combo = [(0, 0), (2, 1), (1, 0), (3, 1)]

            def combine_tail(rsb, pvs, hp, nh):
                # deferred combine tail: -lam scale, PE partition-broadcast of
                # the reciprocals (col-packed 2-per-bank into one borrowed
                # score slot), SBUF staging, and the GPSIMD normalize/merge
                def _tail():
                    nc.gpsimd.tensor_scalar_mul(
                        rsb[0:1, 2:4, :], rsb[0:1, 2:4, :], neglam[:]
                    )
                    rbc_ps = pS.tile([128, 2, 512], f32, tag="s", name="rbcps")
                    for ci in range(4):
                        base = 64 * (ci % 2)
                        nc.tensor.matmul(
                            rbc_ps[base : base + 64, ci // 2, :],
                            ones1[:],
                            rsb[0:1, ci, :],
                            start=True,
                            stop=True,
                            tile_position=(0, base),
                        )
                    # two base-0 SBUF tiles (GPSIMD TT requires matching
                    # base partitions for both SBUF inputs)
                    rbc_a = wpool.tile([64, 2, 512], f32, tag="rbc_a")
                    rbc_b = wpool.tile([64, 2, 512], f32, tag="rbc_b")
                    nc.vector.tensor_copy(rbc_a[:], rbc_ps[0:64, :, :])
                    nc.vector.tensor_copy(rbc_b[:], rbc_ps[64:128, :, :])
                    for par in range(2):
                        # combo ci -> (rbc_b if ci odd else rbc_a)[ci//2]
                        c1, c2 = par, 2 + par
                        r1 = (rbc_b if c1 % 2 else rbc_a)[:, c1 // 2, :]
                        r2 = (rbc_b if c2 % 2 else rbc_a)[:, c2 // 2, :]
                        t0 = wpool.tile([64, 512], f32, tag="t0")
                        t1 = wpool.tile([64, 512], f32, tag="t1")
                        nc.gpsimd.tensor_mul(
                            out=t0[:], in0=pvs[0:64, c1, :], in1=r1
                        )
                        nc.gpsimd.tensor_mul(
                            out=t1[:], in0=pvs[0:64, c2, :], in1=r2
                        )
                        nc.gpsimd.tensor_add(
                            out=oT[
                                par * 64 : (par + 1) * 64,
                                hp,
                                nh * 512 : (nh + 1) * 512,
                            ],
                            in0=t0[:],
                            in1=t1[:],
                        )

                return _tail

            pending = None
            if True:
                qkv_queue = []
                for hp in range(4) if "attn" not in skip else []:
                    if hp < 3 and "qkv" not in skip:
                        qkv_queue = [
                            (hp + 1, t, nh2) for t in range(2) for nh2 in range(2)
                        ]
                    for nh in range(2):
                        pv = pV.tile([65, 4, 512], f32, tag="pv")

                        def emit_pv(mc, etiles):
                            for g in range(2):
                                for i in range(2):
                                    ci = 2 * g + i
                                    _rg, par = combo[ci]
                                    nc.tensor.matmul(
                                        pv[:, ci, :],
                                        vsb[:, mc, 2 * hp + par, :],
                                        etiles[g][:, i, :],
                                        start=(mc == 0),
                                        stop=(mc == 7),
                                    )

                        # software pipeline: PV for chunk mc is emitted after
                        # the scores/exp of chunk mc+1, so the PE never stalls
                        # on the current chunk's exp
                        prev = None
                        for mc in range(8):
                            cur = []
                            for g in range(2):
                                s_ps = pS.tile([128, 2, 512], f32, tag="s")
                                for i in range(2):
                                    rg, _par = combo[2 * g + i]
                                    nc.tensor.matmul(
                                        s_ps[:, i, :],
                                        qkvT[
                                            32 * rg : 32 * rg + 32,
                                            4 + hp,
                                            mc * 128 : (mc + 1) * 128,
                                        ],
                                        qkvT[
                                            32 * rg : 32 * rg + 32,
                                            hp,
                                            nh * 512 : (nh + 1) * 512,
                                        ],
                                        start=True,
                                        stop=True,
                                        tile_position=(32 * rg, 0),
                                    )
                                e_sb = epool.tile([128, 2, 512], b16, tag="e")
                                nc.scalar.activation(
                                    e_sb[:], s_ps[:], Exp, scale=0.125
                                )
                                cur.append(e_sb)
                            if mc == 1 and pending is not None:
                                # flush the previous sweep's deferred combine
                                # tail here, deep inside this sweep's stream,
                                # so its PE broadcast never head-blocks the
                                # next scores at the boundary
                                pending()
                                pending = None
                            if prev is not None:
                                emit_pv(mc - 1, prev)
                            prev = cur
                            if mc in (3, 6) and qkv_queue:
                                # drip-feed next head-pair's q/k projection so
                                # its score-slot borrowing never bursts
                                emit_qkv_unit(*qkv_queue.pop(0), on_act=False)
                        emit_pv(7, prev)
                        # combine head: reciprocal straight from the PSUM
                        # denominator row + pv evacuation (releases the PV
                        # banks for the next sweep).  The rest is deferred.
                        rsb = wpool.tile([1, 4, 512], b16, tag="rsb")
                        with nc.allow_low_precision(
                            reason="bf16 softmax denominator reciprocals"
                        ):
                            nc.vector.reciprocal(rsb[:], pv[64:65, :, :])
                        pvs = wpool.tile([65, 4, 512], f32, tag="pvs")
                        nc.vector.tensor_copy(pvs[:], pv[:])
                        pending = combine_tail(rsb, pvs, hp, nh)

                if pending is not None:
                    pending()
                    pending = None

            # ---------------- Phase C: output projection --------------
            if True:
                for ncc in range(8) if "proj" not in skip else []:
                    for jh in range(2):
                        ps = qkv_psum()
                        for ci in range(4):
                            nc.tensor.matmul(
                                ps[:],
                                oT[:, ci, ncc * 128 : (ncc + 1) * 128],
                                wp[:, ci, jh * 512 : (jh + 1) * 512],
                                start=(ci == 0),
                                stop=(ci == 3),
                            )
                        osb = wpool.tile([128, 512], f32, tag="osb")
                        nc.scalar.copy(osb[:], ps[:])
                        nc.sync.dma_start(
                            out_d[ncc, :, jh * 512 : (jh + 1) * 512], osb[:]
                        )

            loop_ctx.__exit__(None, None, None)

    nc.compile()
    _PROG_CACHE[key] = nc
    return nc


def _prep_core_inputs(x, W_qkv, W_proj, neg_lam):
    """Host-side shard prep. Returns in_maps for the 8 cores."""
    W4 = np.asarray(W_qkv, np.float32).reshape(3, H, HD, C)
    in_maps = []
    for core in range(8):
        b, hg = divmod(core, 2)
        xT = (
            np.ascontiguousarray(np.asarray(x[b], np.float32).T)
            .reshape(8, 128, N)
            .astype(BF16)
        )
        wsl = W4[:, hg * 8 : (hg + 1) * 8]  # [3, 8, 64, 1024]
        wqkv = (
            np.ascontiguousarray(wsl.transpose(3, 0, 1, 2).reshape(C, 1536))
            .reshape(8, 128, 1536)
            .astype(BF16)
        )
        wp = (
            np.ascontiguousarray(
                np.asarray(W_proj, np.float32)[:, hg * 512 : (hg + 1) * 512].T
            )
            .reshape(4, 128, C)
            .astype(BF16)
        )
        in_maps.append(
            {
                "xT": xT,
                "wqkv": wqkv,
                "wp": wp,
                "neglam": np.full((1, 1), neg_lam, np.float32),
            }
        )
    return in_maps


def kernel(x, W_qkv, W_proj, b_proj, lambda_q1, lambda_k1, lambda_q2, lambda_k2):
    from concourse.bass_utils import run_bass_kernel_spmd

    lq1 = np.asarray(lambda_q1, np.float64)
    lk1 = np.asarray(lambda_k1, np.float64)
    lq2 = np.asarray(lambda_q2, np.float64)
    lk2 = np.asarray(lambda_k2, np.float64)
    lam = float(np.mean(np.exp(lq1 * lk1) - np.exp(lq2 * lk2) + LAMBDA_INIT))

    nc = _build_program()
    in_maps = _prep_core_inputs(x, W_qkv, W_proj, -lam)
    res = run_bass_kernel_spmd(nc, in_maps, core_ids=list(range(8)))
    _PROG_CACHE["last_result"] = res

    bp = np.asarray(b_proj, np.float32)
    out = np.empty((B, N, C), np.float32)
    for b in range(B):
        p0 = res.results[2 * b]["out"].reshape(N, C)
        p1 = res.results[2 * b + 1]["out"].reshape(N, C)
        out[b] = p0 + p1 + bp[None, :]
    return out



# revision 13
# speedup vs baseline: 1.1711x; 1.1711x over previous
"""DifferentialAttention Trainium2 kernel (8 NeuronCores, SPMD).

Sharding: data-parallel over batch B=4, tensor-parallel over heads
(2 cores per batch element, 8 heads each).  Each core computes the
partial projection output for its 8 heads; the host sums the two
bf16 partials per batch element in f32 and adds b_proj.

Per-core pipeline (bf16 matmuls, fp32 PSUM):
  1. QKV^T = W_slice^T.T @ x^T            -> [channels, n] layout
  2. V via PE with swapped operands        -> [keys, ch|1] layout
  3. scores S^T[keys, n] per (head, half) with 4-way row groups
  4. exp on ACT (scale=1/8 folded), bf16 out; ACT runs ONLY exp in
     steady state - it is the binding engine (~123us of exp)
  5. PV with stationary-E / moving-[V|1]: out[n, 65] accumulated over
     key chunks; denominator lands per-PARTITION (column 64)
  6. combine on DVE: per-partition reciprocal + scalar_tensor_tensor
     (o1*r1 + o2*(-lam*r2)) -> oc[n, head, ch]
  7. PE transpose (vs identity) oc -> oT[ch, n]; proj = oT.T @ Wp
"""

import sys

sys.path.insert(0, "/opt/trn_rl_repo")

import numpy as np
import ml_dtypes

B, N, C, H, HD = 4, 1024, 1024, 16, 64
LAMBDA_INIT = 0.8
BF16 = ml_dtypes.bfloat16

_PROG_CACHE = {}


def _build_program(loop_n=1, dma_outside=False, skip=(), debug=False):
    key = ("nc", loop_n, dma_outside, tuple(skip), debug)
    if key in _PROG_CACHE:
        return _PROG_CACHE[key]

    import concourse.mybir as mybir
    import concourse.tile as tile
    from concourse import bacc

    f32 = mybir.dt.float32
    b16 = mybir.dt.bfloat16
    Exp = mybir.ActivationFunctionType.Exp
    MUL = mybir.AluOpType.mult
    ADD = mybir.AluOpType.add

    nc = bacc.Bacc(None)

    # host layouts are partition-major so each DMA is one large transfer
    x_d = nc.dram_tensor("xT", [128, 8, N], b16, kind="ExternalInput")
    # wqkv columns reordered: block j'=2*hp+t (t=0 q, t=1 k), v at 1024:1536
    wqkv_d = nc.dram_tensor("wqkv", [128, 8, 1536], b16, kind="ExternalInput")
    wp_d = nc.dram_tensor("wp", [128, 4, C], b16, kind="ExternalInput")
    neglam_d = nc.dram_tensor("neglam", [128, 1], f32, kind="ExternalInput")
    ident_d = nc.dram_tensor("ident", [128, 128], b16, kind="ExternalInput")
    out_d = nc.dram_tensor("out", [8, 128, C], b16, kind="ExternalOutput")
    if debug:
        dbg_qkvT = nc.dram_tensor("dbg_qkvT", [128, 8, N], b16, kind="ExternalOutput")
        dbg_vsb = nc.dram_tensor("dbg_vsb", [128, 8, 8, 65], b16, kind="ExternalOutput")
        dbg_oc = nc.dram_tensor("dbg_oc", [128, 8, 8, 64], b16, kind="ExternalOutput")
        dbg_oT = nc.dram_tensor("dbg_oT", [128, 4, N], b16, kind="ExternalOutput")

    with tile.TileContext(nc) as tc:
        with (
            tc.tile_pool(name="io", bufs=1) as iopool,
            tc.tile_pool(name="work", bufs=4) as wpool,
            tc.tile_pool(name="esb", bufs=10) as epool,
            tc.tile_pool(name="pS", bufs=2, space="PSUM") as pS,
            tc.tile_pool(name="pO", bufs=4, space="PSUM") as pO,
        ):
            xT = iopool.tile([128, 8, N], b16)
            wqkv = iopool.tile([128, 8, 1536], b16)
            wp = iopool.tile([128, 4, C], b16)
            neglam = iopool.tile([128, 1], f32)
            ident = iopool.tile([128, 128], b16)
            # qkvT chunk j=2*hp+t: partitions 0-63 even head d0..63,
            # 64-127 odd head d0..63
            qkvT = iopool.tile([128, 8, N], b16)
            # V in [keys, channels] layout; col 64 of each head = ones
            vsb = iopool.tile([128, 8, 8, 65], b16)
            # combined attention out: [n-part, global n-chunk, head, ch]
            oc = iopool.tile([128, 8, 8, 64], b16)
            # transposed for proj: [ch-part, hp, n]
            oT = iopool.tile([128, 4, N], b16)

            nc.gpsimd.memset(vsb[:, :, :, 64:65], 1.0)
            if "attn" in skip:
                nc.gpsimd.memset(oT[:], 0.0)

            def dma_inputs():
                # transfers serialize globally in issue order; order by need
                for h4 in range(4):
                    nc.sync.dma_start(
                        xT[:, 2 * h4 : 2 * h4 + 2, :], x_d[:, 2 * h4 : 2 * h4 + 2, :]
                    )
                nc.scalar.dma_start(wqkv[:, :, 0:256], wqkv_d[:, :, 0:256])
                nc.gpsimd.dma_start(ident[:], ident_d[:])
                nc.gpsimd.dma_start(neglam[:], neglam_d[:])
                nc.scalar.dma_start(wqkv[:, :, 1024:1536], wqkv_d[:, :, 1024:1536])
                for hp in range(1, 4):
                    c0 = hp * 256
                    nc.scalar.dma_start(
                        wqkv[:, :, c0 : c0 + 256], wqkv_d[:, :, c0 : c0 + 256]
                    )
                nc.sync.dma_start(wp[:], wp_d[:])

            def qkv_psum():
                t = pS.tile([128, 2, 512], f32, tag="s", name="qkvps")
                return t[:, 0, :]

            if dma_outside:
                dma_inputs()
            assert loop_n == 1
            if not dma_outside:
                dma_inputs()

            # ---------------- QKV projection units ---------------------
            def emit_qkv_unit(hp, t, nh, on_act=True):
                # one [128,512] q/k projection chunk -> qkvT[:, 2hp+t, nh]
                evac = nc.scalar.copy if on_act else nc.vector.tensor_copy
                j = 2 * hp + t
                ps = qkv_psum()
                for cc in range(8):
                    nc.tensor.matmul(
                        ps[:],
                        wqkv[:, cc, j * 128 : (j + 1) * 128],
                        xT[:, cc, nh * 512 : (nh + 1) * 512],
                        start=(cc == 0),
                        stop=(cc == 7),
                    )
                evac(qkvT[:, j, nh * 512 : (nh + 1) * 512], ps[:])

            def emit_v_unit(mc):
                # v -> [keys, channels] layout (operands swapped); DVE evac
                ps = qkv_psum()
                for cc in range(8):
                    nc.tensor.matmul(
                        ps[:],
                        xT[:, cc, mc * 128 : (mc + 1) * 128],
                        wqkv[:, cc, 1024:1536],
                        start=(cc == 0),
                        stop=(cc == 7),
                    )
                nc.vector.tensor_copy(
                    vsb[:, mc, :, 0:64], ps.rearrange("p (g d) -> p g d", g=8)
                )

            # phase A: q/k for head-pair 0 (ACT evac - no exps yet)
            if "qkv" not in skip:
                for t in range(2):
                    for nh in range(2):
                        emit_qkv_unit(0, t, nh, on_act=True)

            # ---------------- attention sweeps --------------------------
            # combo ci: 0=(even,h1) 1=(odd,h1) 2=(even,h2) 3=(odd,h2)
            # score row group rg for ci: [0, 2, 1, 3][ci]; parity = ci%2
            RG = [0, 2, 1, 3]
            LAG = 3

            def emit_scores_exp(hp, nh, mc):
                # 4 score matmuls + 2 exps; returns e tiles [g0, g1]
                cur = []
                for g in range(2):
                    s_ps = pS.tile([128, 2, 512], f32, tag="s")
                    for i in range(2):
                        rg = RG[2 * g + i]
                        nc.tensor.matmul(
                            s_ps[:, i, :],
                            qkvT[
                                32 * rg : 32 * rg + 32,
                                2 * hp + 1,
                                mc * 128 : (mc + 1) * 128,
                            ],
                            qkvT[
                                32 * rg : 32 * rg + 32,
                                2 * hp,
                                nh * 512 : (nh + 1) * 512,
                            ],
                            start=True,
                            stop=True,
                            tile_position=(32 * rg, 0),
                        )
                    e_sb = epool.tile([128, 2, 512], b16, tag="e")
                    nc.scalar.activation(e_sb[:], s_ps[:], Exp, scale=0.125)
                    cur.append(e_sb)
                return cur

            def emit_pv(hp, etiles, o_tiles, mc):
                # stationary-E PV: 16 matmuls [128n, 65] accumulating over mc
                # one start/stop per PSUM bank: start=True zeroes the whole
                # 2KB bank, so only the bank's first write may set it
                for g in range(2):
                    for i in range(2):
                        ci = 2 * g + i
                        par = ci % 2
                        for nsub in range(4):
                            nc.tensor.matmul(
                                o_tiles[ci][:, nsub, :],
                                etiles[g][:, i, nsub * 128 : (nsub + 1) * 128],
                                vsb[:, mc, 2 * hp + par, :],
                                start=(mc == 0 and nsub == 0),
                                stop=(mc == 7 and nsub == 3),
                                skip_group_check=True,
                            )

            def emit_combine(hp, nh, o_tiles):
                # per-partition reciprocals, -lam fold, fused combine on DVE
                r = wpool.tile([128, 4, 4, 1], f32, tag="r")
                for ci in range(4):
                    nc.vector.reciprocal(r[:, ci], o_tiles[ci][:, :, 64:65])
                nc.vector.tensor_scalar_mul(r[:, 2:4], r[:, 2:4], neglam[:])
                for par in range(2):
                    ci1, ci2 = par, 2 + par
                    for nsub in range(4):
                        gn = nh * 4 + nsub
                        t = wpool.tile([128, 64], f32, tag=f"t{par}")
                        nc.vector.tensor_scalar_mul(
                            t[:], o_tiles[ci2][:, nsub, 0:64], r[:, ci2, nsub]
                        )
                        nc.vector.scalar_tensor_tensor(
                            oc[:, gn, 2 * hp + par, :],
                            o_tiles[ci1][:, nsub, 0:64],
                            r[:, ci1, nsub],
                            t[:],
                            MUL,
                            ADD,
                        )

            def emit_transposes(hp, nh):
                # oc[n, 2 heads, 64] -> oT[128 ch, n] via PE transpose
                for nsub in range(4):
                    gn = nh * 4 + nsub
                    trb = pO.tile([128, 128], b16, tag="o", name="tr")
                    nc.tensor.matmul(
                        trb[:],
                        oc[:, gn, 2 * hp : 2 * hp + 2, :],
                        ident[:],
                        is_transpose=True,
                    )
                    nc.vector.tensor_copy(
                        oT[:, hp, gn * 128 : (gn + 1) * 128], trb[:]
                    )

            if "attn" not in skip:
                qkv_queue = []
                pending = None  # (hp, nh, etile-list, o_tiles)
                for sweep in range(8):
                    hp, nh = sweep // 2, sweep % 2
                    if nh == 0 and hp < 3 and "qkv" not in skip:
                        qkv_queue = [
                            (hp + 1, t, nh2) for t in range(2) for nh2 in range(2)
                        ]
                    o_tiles = None
                    edeque = []
                    for mc in range(8):
                        edeque.append(emit_scores_exp(hp, nh, mc))
                        if sweep == 0 and "qkv" not in skip:
                            emit_v_unit(mc)
                        if mc == 1 and pending is not None:
                            # previous sweep's combine + transposes, placed
                            # after this sweep's pipeline is warmed up
                            emit_combine(*pending)
                            emit_transposes(pending[0], pending[1])
                            pending = None
                        if mc >= LAG:
                            if o_tiles is None:
                                # allocated after the previous sweep's
                                # transposes so the slot ring hands off
                                # ci -> tr -> next ci without deadlock
                                o_tiles = [
                                    pO.tile(
                                        [128, 4, 65], f32, tag="o",
                                        name=f"o{sweep}_{ci}",
                                    )
                                    for ci in range(4)
                                ]
                            emit_pv(hp, edeque[mc - LAG], o_tiles, mc - LAG)
                        if mc in (3, 6) and qkv_queue:
                            emit_qkv_unit(*qkv_queue.pop(0), on_act=False)
                    for mc in range(max(0, 8 - LAG), 8):
                        if o_tiles is None:
                            o_tiles = [
                                pO.tile(
                                    [128, 4, 65], f32, tag="o",
                                    name=f"o{sweep}_{ci}",
                                )
                                for ci in range(4)
                            ]
                        emit_pv(hp, edeque[mc], o_tiles, mc)
                    pending = (hp, nh, o_tiles)

                if pending is not None:
                    emit_combine(*pending)
                    emit_transposes(pending[0], pending[1])
                    pending = None

            if debug:
                nc.sync.dma_start(dbg_qkvT[:], qkvT[:])
                nc.sync.dma_start(dbg_vsb[:], vsb[:])
                nc.sync.dma_start(dbg_oc[:], oc[:])
                nc.sync.dma_start(dbg_oT[:], oT[:])

            # ---------------- output projection -------------------------
            if "proj" not in skip:
                for ncc in range(8):
                    for jh in range(2):
                        ps = qkv_psum()
                        for ci in range(4):
                            nc.tensor.matmul(
                                ps[:],
                                oT[:, ci, ncc * 128 : (ncc + 1) * 128],
                                wp[:, ci, jh * 512 : (jh + 1) * 512],
                                start=(ci == 0),
                                stop=(ci == 3),
                            )
                        osb = wpool.tile([128, 512], b16, tag="osb")
                        nc.scalar.copy(osb[:], ps[:])
                        nc.sync.dma_start(
                            out_d[ncc, :, jh * 512 : (jh + 1) * 512], osb[:]
                        )

    nc.compile()
    _PROG_CACHE[key] = nc
    return nc


def _prep_core_inputs(x, W_qkv, W_proj, neg_lam):
    """Host-side shard prep. Returns in_maps for the 8 cores."""
    W4 = np.asarray(W_qkv, np.float32).reshape(3, H, HD, C)
    ident = np.eye(128, dtype=np.float32).astype(BF16)
    in_maps = []
    for core in range(8):
        b, hg = divmod(core, 2)
        xT = (
            np.ascontiguousarray(np.asarray(x[b], np.float32).T)
            .reshape(8, 128, N)
            .transpose(1, 0, 2)
            .astype(BF16)
        )
        wsl = W4[:, hg * 8 : (hg + 1) * 8]  # [3, 8 heads, 64, 1024]
        # columns [t(3), head(8), d(64)]; reorder q/k to j'=2*hp+t blocks
        Wcols = np.ascontiguousarray(wsl.transpose(3, 0, 1, 2).reshape(C, 1536))
        Wnew = np.empty_like(Wcols)
        for hp in range(4):
            for t in range(2):
                src = t * 512 + hp * 128
                dst = (2 * hp + t) * 128
                Wnew[:, dst : dst + 128] = Wcols[:, src : src + 128]
        Wnew[:, 1024:1536] = Wcols[:, 1024:1536]
        wqkv = Wnew.reshape(8, 128, 1536).transpose(1, 0, 2).astype(BF16)
        wp = (
            np.ascontiguousarray(
                np.asarray(W_proj, np.float32)[:, hg * 512 : (hg + 1) * 512].T
            )
            .reshape(4, 128, C)
            .transpose(1, 0, 2)
            .astype(BF16)
        )
        in_maps.append(
            {
                "xT": np.ascontiguousarray(xT),
                "wqkv": np.ascontiguousarray(wqkv),
                "wp": np.ascontiguousarray(wp),
                "neglam": np.full((128, 1), neg_lam, np.float32),
                "ident": ident,
            }
        )
    return in_maps


def kernel(x, W_qkv, W_proj, b_proj, lambda_q1, lambda_k1, lambda_q2, lambda_k2):
    from concourse.bass_utils import run_bass_kernel_spmd

    lq1 = np.asarray(lambda_q1, np.float64)
    lk1 = np.asarray(lambda_k1, np.float64)
    lq2 = np.asarray(lambda_q2, np.float64)
    lk2 = np.asarray(lambda_k2, np.float64)
    lam = float(np.mean(np.exp(lq1 * lk1) - np.exp(lq2 * lk2) + LAMBDA_INIT))

    nc = _build_program()
    in_maps = _prep_core_inputs(x, W_qkv, W_proj, -lam)
    res = run_bass_kernel_spmd(nc, in_maps, core_ids=list(range(8)))
    _PROG_CACHE["last_result"] = res

    bp = np.asarray(b_proj, np.float32)
    out = np.empty((B, N, C), np.float32)
    for b in range(B):
        p0 = res.results[2 * b]["out"].astype(np.float32).reshape(N, C)
        p1 = res.results[2 * b + 1]["out"].astype(np.float32).reshape(N, C)
        out[b] = p0 + p1 + bp[None, :]
    return out
